# revision 15
# baseline (speedup 1.0000x reference)
"""Antialiased 2x upsampling (StyleGAN2 upsample_2d, k=[1,3,3,1], factor=2).

Input  x: (8, 256, 256, 64) f32 NHWC  ->  output: (8, 511, 511, 64) f32.

Math (separable, polyphase):
  g[i] = x[i-1]/3 + x[i]   (even out row 2i),  h[i] = x[i]/3 + x[i-1] (odd 2i-1)
  out[2i,   2j]   = 9/16*g[j]   + 3/16*g[j-1]
  out[2i,   2j-1] = 9/16*g[j-1] + 3/16*g[j]     (same for h on odd rows)

Sharding: pure data parallel, one batch image per NeuronCore (8 cores).

v2 design (TensorEngine row-pass):
- x is loaded ONCE per tile (128 rows incl. a 1-row halo) as bf16.
- The H-pass AND the 9/16 pre-scale are banded [128->127] matmuls on the
  idle TensorEngine: c9 = W9^T B with W9[q,p] = 3/16 d(q,p) + 9/16 d(q,p+1)
  (g block) resp. the h block. Weights exact in bf16; PSUM accumulates f32.
  This removes the baseline's second (row-shifted) HBM read of x (~17MB/core)
  and the DVE H-pass work.
- The idle scalar engine (ACT) derives c3 = c9/3 from PSUM into SBUF f32
  (the ISA forbids DVE tensor_tensor with BOTH operands in PSUM).
- DVE only does the W-pass: one tensor_add per output element (c9 from
  PSUM + shifted c3 from SBUF), writing the interleaved bf16 rowbuf.
- PSUM chunking: 1 bank (512 f32 = 8 g-cols: 1 halo + 7 new) per chunk,
  tag p9 bufs=8 -> all 8 banks, PE runs chunks ahead of ACT/DVE.
- WT=128 -> store packets are 64KB f32 per partition-row (2x baseline).
- Edge rows (out 0, 509, 510) are a 3-partition pass using a small 3x3
  weight block, scattered through the main loop in 4 w-quarters.
"""

import numpy as np

import concourse.bacc as bacc
import concourse.mybir as mybir
from concourse.tile import TileContext
from concourse.bass_utils import run_bass_kernel_spmd

F32 = mybir.dt.float32
BF16 = mybir.dt.bfloat16

B_FULL, H_FULL, W_FULL, C_FULL = 8, 256, 256, 64
N_CORES = 8


def make_weights():
    """W9: [128, 257] f32. cols 0:127 g-block, 127:254 h-block, 254:257 edge."""
    w9 = np.zeros((128, 257), dtype=np.float32)
    for p in range(127):
        # g9[p] = 3/16 x[i-1] + 9/16 x[i] = 3/16 B[p] + 9/16 B[p+1]
        w9[p, p] = 3.0 / 16.0
        w9[p + 1, p] = 9.0 / 16.0
        # h9[p] = 9/16 B[p] + 3/16 B[p+1]
        w9[p, 127 + p] = 9.0 / 16.0
        w9[p + 1, 127 + p] = 3.0 / 16.0
    # edge: partitions {x[254], x[255], x[0]} -> rows {509 (h@255), 510 (g@255), 0 (g@0)}
    w9[0, 254] = 9.0 / 16.0  # h9[255] = 3/16 x[255] + 9/16 x[254]
    w9[1, 254] = 3.0 / 16.0
    w9[0, 255] = 3.0 / 16.0  # g9[255] = 3/16 x[254] + 9/16 x[255]
    w9[1, 255] = 9.0 / 16.0
    w9[2, 256] = 9.0 / 16.0  # g9[0] = 9/16 x[0]   (x[-1] = 0)
    return w9


def build_upsample_tile(tc, out, x, w9d, H, W, C):
    nc = tc.nc
    WT = 128
    n_wt = W // WT
    FW = (WT + 1) * C          # 8256: halo col w0-1 plus WT cols
    seg = 2 * WT * C           # 16384: one output-row segment (2*WT out cols)
    PT = 127                   # out rows per h-tile (B tile holds PT+1 = 128 rows)
    n_ht = 2
    assert n_ht * PT == H - 2  # main tiles: i = 1..254 (out rows 1..508)
    # edge pass covers out rows 0, 509, 510

    NCH = 7                    # new g-cols per psum chunk (1 bank = 8 cols w/ halo)
    EQ = 4                     # edge pass split into 4 w-quarters of 64 cols
    EW = W // EQ               # 64
    eFW = (EW + 1) * C         # 4160
    eseg = 2 * EW * C          # 8192

    with (
        tc.tile_pool(name="io", bufs=2) as io_pool,
        tc.tile_pool(name="rb", bufs=2) as rb_pool,
        tc.tile_pool(name="ep", bufs=1) as ep_pool,
        tc.tile_pool(name="cst", bufs=1) as cst_pool,
        tc.tile_pool(name="s3", bufs=4) as s3_pool,
        tc.tile_pool(name="ps", bufs=8, space="PSUM") as ps_pool,
    ):
        # ---- weights -> SBUF (bf16; all values exact)
        w9s = cst_pool.tile([128, 257], BF16, tag="w9", name="w9s")
        nc.gpsimd.dma_start(out=w9s[:], in_=w9d[:, :])

        def pchunks():
            return [(0, 64), (64, 127)]

        # ---------- main tiles ----------
        def load(s):
            t, wt = s // n_wt, s % n_wt
            r0 = 127 * t                     # B rows r0 .. r0+127
            Bt = io_pool.tile([128, FW], BF16, tag="B", name=f"B_{t}_{wt}")
            if wt == 0:
                nc.vector.memset(Bt[:, 0:C], 0.0)
                lo = C
            else:
                lo = 0
            cl = (wt * WT - 1) * C           # x col offset of tile col 0
            for q0, q1 in ((0, 64), (64, 128)):
                nc.gpsimd.dma_start(
                    out=Bt[q0:q1, lo:FW],
                    in_=x[r0 + q0 : r0 + q1, cl + lo : cl + FW],
                )
            return Bt

        def chunk_ops(Bt, rbv, k, jlo, nj):
            """One psum chunk: g-cols jlo..jlo+nj (tile-local new cols jlo..)."""
            ne = (nj + 1) * C                # psum elems incl halo col
            win = Bt[:, jlo * C : jlo * C + ne]
            for s_seg, wofs in ((1, 0), (0, 127)):   # even rows from g, odd from h
                P9 = ps_pool.tile([128, 512], F32, tag="p9", name=f"p9_{k}_{s_seg}")
                S3 = s3_pool.tile([128, 512], F32, tag="s3", name=f"s3_{k}_{s_seg}")
                nc.tensor.matmul(P9[:PT, :ne], w9s[:, wofs : wofs + PT], win)
                nc.scalar.mul(S3[:PT, :ne], P9[:PT, :ne], 1.0 / 3.0)
                # out col 2w   (q=1): 9/16 c[w]   + 3/16 c[w-1]
                nc.vector.tensor_add(
                    out=rbv[:PT, s_seg, jlo : jlo + nj, 1, :],
                    in0=P9[:PT, C : C + nj * C],
                    in1=S3[:PT, 0 : nj * C],
                )
                # out col 2w-1 (q=0): 9/16 c[w-1] + 3/16 c[w]
                nc.vector.tensor_add(
                    out=rbv[:PT, s_seg, jlo : jlo + nj, 0, :],
                    in0=P9[:PT, 0 : nj * C],
                    in1=S3[:PT, C : C + nj * C],
                )

        def compute(s, Bt, edge_hook):
            t, wt = s // n_wt, s % n_wt
            rb = rb_pool.tile([128, 2 * seg], BF16, tag="rb", name=f"rb_{t}_{wt}")
            rbv = rb.rearrange("p (s j q c) -> p s j q c", s=2, j=WT, q=2, c=C)
            n_chunks = (WT + NCH - 1) // NCH
            for k in range(n_chunks):
                jlo = k * NCH
                nj = min(NCH, WT - jlo)
                chunk_ops(Bt, rbv, k, jlo, nj)
                if edge_hook is not None and k == 4:
                    edge_hook()
            return rb

        def store(s, rb):
            t, wt = s // n_wt, s % n_wt
            i0 = 1 + 127 * t
            skip = C if wt == 0 else 0
            dcol = 0 if wt == 0 else (2 * wt * WT - 1) * C
            dw = seg - skip
            for q0, q1 in pchunks():
                r0 = 2 * (i0 + q0) - 1
                nc.gpsimd.dma_start(
                    out=out[r0 : r0 + 2 * (q1 - q0) - 1 : 2, dcol : dcol + dw],
                    in_=rb[q0:q1, skip:seg],
                )
            for q0, q1 in pchunks():
                r0 = 2 * (i0 + q0)
                nc.gpsimd.dma_start(
                    out=out[r0 : r0 + 2 * (q1 - q0) - 1 : 2, dcol : dcol + dw],
                    in_=rb[q0:q1, seg + skip : 2 * seg],
                )

        # ---------- edge pass (out rows 509, 510, 0) in 4 w-quarters ----------
        def edge_load(wq):
            Be = ep_pool.tile([3, eFW], BF16, tag="Be", name=f"Be_{wq}")
            if wq == 0:
                nc.vector.memset(Be[:, 0:C], 0.0)
                lo = C
            else:
                lo = 0
            cl = (wq * EW - 1) * C
            nc.gpsimd.dma_start(out=Be[0:2, lo:eFW], in_=x[254:256, cl + lo : cl + eFW])
            nc.gpsimd.dma_start(out=Be[2:3, lo:eFW], in_=x[0:1, cl + lo : cl + eFW])
            return Be

        def edge_compute(wq, Be):
            rbe = ep_pool.tile([3, eseg], BF16, tag="rbe", name=f"rbe_{wq}")
            rbev = rbe.rearrange("p (j q c) -> p j q c", j=EW, q=2, c=C)
            n_chunks = (EW + NCH - 1) // NCH
            for k in range(n_chunks):
                jlo = k * NCH
                nj = min(NCH, EW - jlo)
                ne = (nj + 1) * C
                win = Be[:3, jlo * C : jlo * C + ne]
                E9 = ps_pool.tile([128, 512], F32, tag="p9", name=f"e9_{wq}_{k}")
                S3 = s3_pool.tile([128, 512], F32, tag="s3", name=f"es3_{wq}_{k}")
                nc.tensor.matmul(E9[:3, :ne], w9s[0:3, 254:257], win)
                nc.scalar.mul(S3[:3, :ne], E9[:3, :ne], 1.0 / 3.0)
                nc.vector.tensor_add(
                    out=rbev[:3, jlo : jlo + nj, 1, :],
                    in0=E9[:3, C : C + nj * C],
                    in1=S3[:3, 0 : nj * C],
                )
                nc.vector.tensor_add(
                    out=rbev[:3, jlo : jlo + nj, 0, :],
                    in0=E9[:3, 0 : nj * C],
                    in1=S3[:3, C : C + nj * C],
                )
            return rbe

        def edge_store(wq, rbe):
            skip = C if wq == 0 else 0
            dcol = 0 if wq == 0 else (2 * wq * EW - 1) * C
            dw = eseg - skip
            nc.gpsimd.dma_start(
                out=out[509:511, dcol : dcol + dw], in_=rbe[0:2, skip:eseg]
            )
            nc.gpsimd.dma_start(
                out=out[0:1, dcol : dcol + dw], in_=rbe[2:3, skip:eseg]
            )

        # ---------- pipeline ----------
        N = n_ht * n_wt                      # 4 main steps
        PRE = 2
        btiles = {}
        for s in range(min(PRE, N)):
            btiles[s] = load(s)
        ebuf = {"B": edge_load(0), "rb": None, "wq": 0}

        def edge_hook_step(s):
            if s >= EQ:
                return None

            def hook():
                wq = s
                rbe = edge_compute(wq, ebuf["B"])
                ebuf["rb"] = rbe
                if wq + 1 < EQ:
                    ebuf["B"] = edge_load(wq + 1)
            return hook

        for s in range(N):
            if s + PRE < N:
                btiles[s + PRE] = load(s + PRE)
            rb = compute(s, btiles.pop(s), edge_hook_step(s))
            store(s, rb)
            if ebuf["rb"] is not None:
                edge_store(s, ebuf["rb"])
                ebuf["rb"] = None


def build_nc(H=H_FULL, W=W_FULL, C=C_FULL):
    nc = bacc.Bacc(
        "TRN2", target_bir_lowering=False, debug=False,
        dynamic_dma_scratch_size=16384,
    )
    x = nc.declare_dram_parameter("x", [H, W * C], F32, isOutput=False).ap()
    w9d = nc.declare_dram_parameter("w9", [128, 257], F32, isOutput=False).ap()
    out = nc.declare_dram_parameter(
        "out", [2 * H - 1, (2 * W - 1) * C], F32, isOutput=True
    ).ap()
    with TileContext(nc) as tc:
        build_upsample_tile(tc, out, x, w9d, H, W, C)
    nc.compile()
    return nc


_NC_CACHE = {}


def _get_nc():
    key = (H_FULL, W_FULL, C_FULL)
    if key not in _NC_CACHE:
        _NC_CACHE[key] = build_nc()
    return _NC_CACHE[key]


def run_spmd(x, trace=False, **kwargs):
    """x: (8, 256, 256, 64) f32. Returns (BassKernelResults, out (8,511,511,64))."""
    nc = _get_nc()
    w9 = make_weights()
    in_maps = [
        {
            "x": np.ascontiguousarray(x[b]).reshape(H_FULL, W_FULL * C_FULL),
            "w9": w9,
        }
        for b in range(N_CORES)
    ]
    res = run_bass_kernel_spmd(
        nc, in_maps, core_ids=list(range(N_CORES)), trace=trace, **kwargs
    )
    out = np.stack(
        [
            res.results[b]["out"].reshape(2 * H_FULL - 1, 2 * W_FULL - 1, C_FULL)
            for b in range(N_CORES)
        ]
    )
    return res, out


def kernel(x):
    x = np.asarray(x, dtype=np.float32)
    _, out = run_spmd(x, trace=False)
    return out
